# revision 1
# baseline (speedup 1.0000x reference)
"""Trainium2 Bass kernel for nn_ConceptDiagram (moe_routing).

Reference computation (per item i with source type s = type_ids[i]):
    t* = argmax_t type_matching[s, t]
    h   = relu(states @ W1[s,t*] + b1[s,t*]);  y = h @ W2[s,t*] + b2[s,t*]
    hc  = relu(states @ C1[s,t*] + c1[s,t*]);  cls = sigmoid(hc @ C2[s,t*] + c2[s,t*])
    out_state[i] = y[i];  out_score[i] = min(scores[i], cls[i])
    item_prob[i] = sigmoid(type_matching[s, t*])

Sharding strategy: the items are routed by source type (the MoE routing
step); type s is assigned to cores {2s, 2s+1}, each taking half of that
type's items.  Each core receives its shard of `states` already laid out
transposed ([D, n] with D on partitions, padded to a whole number of
512-item tiles) plus the single expert's weights, and computes the full
mapper + classifier chain on device.  Outputs are returned per-shard and
scattered back to the full arrays (unsharding).

Matmuls run in float32r (full-rate fp32 PE mode, ~1e-4 relative error);
all accumulation is fp32 in PSUM.
"""

import sys

if "/opt/trn_rl_repo" not in sys.path:
    sys.path.insert(0, "/opt/trn_rl_repo")

import numpy as np

import concourse.bass as bass  # noqa: F401
import concourse.mybir as mybir
import concourse.tile as tile
from concourse import bacc
from concourse.bass_utils import run_bass_kernel_spmd

F32 = mybir.dt.float32
F32R = mybir.dt.float32r
RELU = mybir.ActivationFunctionType.Relu
SIGMOID = mybir.ActivationFunctionType.Sigmoid

S, T_TYPES, N, D, H, DT = 4, 4, 65536, 128, 128, 128
P = 128
TILE = 512  # items per matmul (one fp32 PSUM bank)
GROUP = 4  # tiles per classifier/score group
N_CORES = 8


def build_bass(n_tiles: int):
    """Per-core kernel: n_tiles tiles of TILE items, one expert."""
    ng = (n_tiles + GROUP - 1) // GROUP
    npad = n_tiles * TILE
    nsc = ng * TILE

    nc = bacc.Bacc(None, target_bir_lowering=False)
    statesT = nc.dram_tensor("statesT", [P, npad], F32R, kind="ExternalInput")
    scoresG = nc.dram_tensor("scoresG", [GROUP, nsc], F32, kind="ExternalInput")
    w1 = nc.dram_tensor("w1", [D, H], F32R, kind="ExternalInput")
    w2 = nc.dram_tensor("w2", [H, DT], F32R, kind="ExternalInput")
    c1m = nc.dram_tensor("c1m", [D, H], F32R, kind="ExternalInput")
    # C2 padded: column 3 holds C2[s,t*], other 6 columns zero; slice
    # [:, 3-j:7-j] puts C2 in local column j -> z lands on psum partition j.
    c2p = nc.dram_tensor("c2p", [H, GROUP + 3], F32R, kind="ExternalInput")
    b1c = nc.dram_tensor("b1c", [H, 1], F32, kind="ExternalInput")
    c1c = nc.dram_tensor("c1c", [H, 1], F32, kind="ExternalInput")
    b2c = nc.dram_tensor("b2c", [DT, 1], F32, kind="ExternalInput")
    c2c = nc.dram_tensor("c2c", [GROUP, 1], F32, kind="ExternalInput")
    tm = nc.dram_tensor("tm", [S, T_TYPES], F32, kind="ExternalInput")

    outT = nc.dram_tensor("outT", [DT, npad], F32, kind="ExternalOutput")
    oscore = nc.dram_tensor("oscore", [GROUP, nsc], F32, kind="ExternalOutput")
    probs = nc.dram_tensor("probs", [S, 1], F32, kind="ExternalOutput")

    with tile.TileContext(nc) as tc:
        with (
            tc.tile_pool(name="const", bufs=1) as const,
            tc.tile_pool(name="xp", bufs=3) as xp,
            tc.tile_pool(name="hp", bufs=2) as hp,
            tc.tile_pool(name="yp", bufs=3) as yp,
            tc.tile_pool(name="sp", bufs=2) as sp,
            tc.tile_pool(name="ps", bufs=2, space="PSUM") as ps,
            tc.tile_pool(name="psz", bufs=2, space="PSUM") as psz,
        ):
            w1_sb = const.tile([D, H], F32R, tag="w1")
            nc.sync.dma_start(out=w1_sb, in_=w1[:])
            w2_sb = const.tile([H, DT], F32R, tag="w2")
            nc.sync.dma_start(out=w2_sb, in_=w2[:])
            c1_sb = const.tile([D, H], F32R, tag="c1m")
            nc.sync.dma_start(out=c1_sb, in_=c1m[:])
            c2_sb = const.tile([H, GROUP + 3], F32R, tag="c2p")
            nc.sync.dma_start(out=c2_sb, in_=c2p[:])
            b1_sb = const.tile([H, 1], F32, tag="b1c")
            nc.sync.dma_start(out=b1_sb, in_=b1c[:])
            c1c_sb = const.tile([H, 1], F32, tag="c1c")
            nc.sync.dma_start(out=c1c_sb, in_=c1c[:])
            b2_sb = const.tile([DT, 1], F32, tag="b2c")
            nc.sync.dma_start(out=b2_sb, in_=b2c[:])
            c2c_sb = const.tile([GROUP, 1], F32, tag="c2c")
            nc.sync.dma_start(out=c2c_sb, in_=c2c[:])
            sc_all = const.tile([GROUP, nsc], F32, tag="sc")
            nc.sync.dma_start(out=sc_all, in_=scoresG[:])
            os_all = const.tile([GROUP, nsc], F32, tag="os")

            # item_prob values: sigmoid(max_t type_matching[s, t]) per type
            tm_sb = const.tile([S, T_TYPES], F32, tag="tm")
            nc.sync.dma_start(out=tm_sb, in_=tm[:])
            rmax = const.tile([S, 1], F32, tag="rmax")
            nc.vector.reduce_max(rmax[:], tm_sb[:], axis=mybir.AxisListType.X)
            prob_sb = const.tile([S, 1], F32, tag="prob")
            nc.scalar.activation(out=prob_sb, in_=rmax, func=SIGMOID)
            nc.sync.dma_start(out=probs[:], in_=prob_sb)

            for g in range(ng):
                k = min(GROUP, n_tiles - g * GROUP)
                x_sb = xp.tile([P, GROUP * TILE], F32R, tag="x")
                nc.sync.dma_start(
                    out=x_sb[:, : k * TILE],
                    in_=statesT[:, g * GROUP * TILE : (g * GROUP + k) * TILE],
                )
                y_sb = yp.tile([DT, GROUP * TILE], F32, tag="y")
                z_ps = psz.tile([GROUP, TILE], F32, tag="z")
                for j in range(k):
                    xt = x_sb[:, j * TILE : (j + 1) * TILE]
                    h_ps = ps.tile([H, TILE], F32, tag="h")
                    nc.tensor.matmul(h_ps, w1_sb, xt, start=True, stop=True)
                    h_sb = hp.tile([H, TILE], F32R, tag="h")
                    nc.scalar.activation(out=h_sb, in_=h_ps, func=RELU, bias=b1_sb)
                    y_ps = ps.tile([DT, TILE], F32, tag="yps")
                    nc.tensor.matmul(y_ps, w2_sb, h_sb, start=True, stop=True)
                    nc.vector.tensor_scalar_add(
                        y_sb[:, j * TILE : (j + 1) * TILE], y_ps, b2_sb
                    )
                    hc_ps = ps.tile([H, TILE], F32, tag="hc")
                    nc.tensor.matmul(hc_ps, c1_sb, xt, start=True, stop=True)
                    hc_sb = hp.tile([H, TILE], F32R, tag="hc")
                    nc.scalar.activation(out=hc_sb, in_=hc_ps, func=RELU, bias=c1c_sb)
                    nc.tensor.matmul(
                        z_ps,
                        c2_sb[:, GROUP - 1 - j : 2 * GROUP - 1 - j],
                        hc_sb,
                        start=(j == 0),
                        stop=(j == k - 1),
                    )
                nc.sync.dma_start(
                    out=outT[:, g * GROUP * TILE : (g * GROUP + k) * TILE],
                    in_=y_sb[:, : k * TILE],
                )
                zs_sb = sp.tile([GROUP, TILE], F32, tag="zs")
                nc.scalar.activation(out=zs_sb, in_=z_ps, func=SIGMOID, bias=c2c_sb)
                nc.vector.tensor_tensor(
                    os_all[:, g * TILE : (g + 1) * TILE],
                    zs_sb,
                    sc_all[:, g * TILE : (g + 1) * TILE],
                    mybir.AluOpType.min,
                )
            nc.sync.dma_start(out=oscore[:], in_=os_all)

    nc.finalize()
    return nc


def _shard(states, scores, type_ids, type_matching, W1, b1, W2, b2, C1, c1, C2, c2):
    """Host-side sharding: route items by type to cores, build per-core inputs."""
    tm = np.ascontiguousarray(np.asarray(type_matching, dtype=np.float32))
    best_t = np.argmax(tm, axis=1)
    tids = np.asarray(type_ids)
    states = np.asarray(states, dtype=np.float32)
    scores_flat = np.asarray(scores, dtype=np.float32).reshape(-1)

    core_idx = []
    for s in range(S):
        idx = np.flatnonzero(tids == s)
        h = (len(idx) + 1) // 2
        core_idx.append(idx[:h])
        core_idx.append(idx[h:])

    n_tiles = max(1, max((len(ci) + TILE - 1) // TILE for ci in core_idx))
    ng = (n_tiles + GROUP - 1) // GROUP
    npad = n_tiles * TILE
    nsc = ng * TILE

    in_maps = []
    for c in range(N_CORES):
        s = c // 2
        t = int(best_t[s])
        ci = core_idx[c]
        cip = np.zeros(npad, dtype=np.int64)
        cip[: len(ci)] = ci
        stT = np.ascontiguousarray(states[cip].T)
        scp = np.zeros(GROUP * nsc, dtype=np.float32)
        scp[:npad] = scores_flat[cip]
        scG = np.ascontiguousarray(
            scp.reshape(ng, GROUP, TILE).transpose(1, 0, 2).reshape(GROUP, nsc)
        )
        c2pad = np.zeros((H, GROUP + 3), dtype=np.float32)
        c2pad[:, GROUP - 1] = np.asarray(C2[s, t], dtype=np.float32).reshape(-1)
        in_maps.append(
            {
                "statesT": stT,
                "scoresG": scG,
                "w1": np.ascontiguousarray(np.asarray(W1[s, t], dtype=np.float32)),
                "w2": np.ascontiguousarray(np.asarray(W2[s, t], dtype=np.float32)),
                "c1m": np.ascontiguousarray(np.asarray(C1[s, t], dtype=np.float32)),
                "c2p": c2pad,
                "b1c": np.asarray(b1[s, t], dtype=np.float32).reshape(H, 1),
                "c1c": np.asarray(c1[s, t], dtype=np.float32).reshape(H, 1),
                "b2c": np.asarray(b2[s, t], dtype=np.float32).reshape(DT, 1),
                "c2c": np.full(
                    (GROUP, 1), np.float32(np.asarray(c2[s, t]).reshape(())),
                    dtype=np.float32,
                ),
                "tm": tm,
            }
        )
    return in_maps, core_idx, n_tiles


def _unshard(results, core_idx, n_tiles):
    ng = (n_tiles + GROUP - 1) // GROUP
    nsc = ng * TILE
    out_state = np.zeros((N, DT), dtype=np.float32)
    out_score = np.zeros((N, 1), dtype=np.float32)
    item_prob = np.zeros((N,), dtype=np.float32)
    for c in range(N_CORES):
        ci = core_idx[c]
        n = len(ci)
        if n == 0:
            continue
        r = results[c]
        out_state[ci] = r["outT"][:, :n].T
        osg = (
            r["oscore"].reshape(GROUP, ng, TILE).transpose(1, 0, 2).reshape(-1)[:n]
        )
        out_score[ci, 0] = osg
        item_prob[ci] = r["probs"][c // 2, 0]
    return out_state, out_score, item_prob


_NC_CACHE: dict[int, object] = {}


def kernel(states, scores, type_ids, type_matching, W1, b1, W2, b2, C1, c1, C2, c2):
    in_maps, core_idx, n_tiles = _shard(
        states, scores, type_ids, type_matching, W1, b1, W2, b2, C1, c1, C2, c2
    )
    nc = _NC_CACHE.get(n_tiles)
    if nc is None:
        nc = build_bass(n_tiles)
        _NC_CACHE[n_tiles] = nc
    res = run_bass_kernel_spmd(nc, in_maps, core_ids=list(range(N_CORES)))
    return _unshard(res.results, core_idx, n_tiles)


# revision 4
# speedup vs baseline: 95.2094x; 95.2094x over previous
"""Trainium2 Bass kernel for nn_ConceptDiagram (moe_routing).

Reference computation (per item i with source type s = type_ids[i]):
    t* = argmax_t type_matching[s, t]
    h   = relu(states @ W1[s,t*] + b1[s,t*]);  y = h @ W2[s,t*] + b2[s,t*]
    hc  = relu(states @ C1[s,t*] + c1[s,t*]);  cls = sigmoid(hc @ C2[s,t*] + c2[s,t*])
    out_state[i] = y[i];  out_score[i] = min(scores[i], cls[i])
    item_prob[i] = sigmoid(type_matching[s, t*])

Sharding strategy: the items are routed by source type (the MoE routing
step); type s is assigned to cores {2s, 2s+1}, each taking half of that
type's items.  Each core receives its shard of `states` already laid out
transposed ([D, n] with D on partitions, padded to a whole number of
512-item tiles) plus the single expert's weights, and computes the full
mapper + classifier chain on device.  Outputs are returned per-shard and
scattered back to the full arrays (unsharding).

Matmuls run in float32r (full-rate fp32 PE mode, ~1e-4 relative error);
all accumulation is fp32 in PSUM.
"""

import sys

if "/opt/trn_rl_repo" not in sys.path:
    sys.path.insert(0, "/opt/trn_rl_repo")

import numpy as np

import concourse.bass as bass  # noqa: F401
import concourse.mybir as mybir
import concourse.tile as tile
from concourse import bacc
from concourse.bass_utils import run_bass_kernel_spmd

F32 = mybir.dt.float32
F32R = mybir.dt.float32r
RELU = mybir.ActivationFunctionType.Relu
SIGMOID = mybir.ActivationFunctionType.Sigmoid

S, T_TYPES, N, D, H, DT = 4, 4, 65536, 128, 128, 128
P = 128
TILE = 512  # items per matmul (one fp32 PSUM bank)
GROUP = 4  # tiles per classifier/score group
N_CORES = 8


def build_bass(n_tiles: int, repeat: int = 1):
    """Per-core kernel: n_tiles tiles of TILE items, one expert.

    repeat > 1 re-runs the whole body (for HW timing by differencing)."""
    ng = (n_tiles + GROUP - 1) // GROUP
    npad = n_tiles * TILE
    nsc = ng * TILE

    nc = bacc.Bacc(None, target_bir_lowering=False)
    statesT = nc.dram_tensor("statesT", [P, npad], F32R, kind="ExternalInput")
    scoresG = nc.dram_tensor("scoresG", [GROUP, nsc], F32, kind="ExternalInput")
    w1 = nc.dram_tensor("w1", [D, H], F32R, kind="ExternalInput")
    w2 = nc.dram_tensor("w2", [H, DT], F32R, kind="ExternalInput")
    c1m = nc.dram_tensor("c1m", [D, H], F32R, kind="ExternalInput")
    # C2 padded: column 3 holds C2[s,t*], other 6 columns zero; slice
    # [:, 3-j:7-j] puts C2 in local column j -> z lands on psum partition j.
    c2p = nc.dram_tensor("c2p", [H, GROUP + 3], F32R, kind="ExternalInput")
    b1c = nc.dram_tensor("b1c", [H, 1], F32, kind="ExternalInput")
    c1c = nc.dram_tensor("c1c", [H, 1], F32, kind="ExternalInput")
    b2c = nc.dram_tensor("b2c", [DT, 1], F32, kind="ExternalInput")
    c2c = nc.dram_tensor("c2c", [GROUP, 1], F32, kind="ExternalInput")
    tm = nc.dram_tensor("tm", [S, T_TYPES], F32, kind="ExternalInput")

    outT = nc.dram_tensor("outT", [DT, npad], F32, kind="ExternalOutput")
    oscore = nc.dram_tensor("oscore", [GROUP, nsc], F32, kind="ExternalOutput")
    probs = nc.dram_tensor("probs", [S, 1], F32, kind="ExternalOutput")

    with tile.TileContext(nc) as tc:
        with (
            tc.tile_pool(name="const", bufs=1) as const,
            tc.tile_pool(name="xp", bufs=3) as xp,
            tc.tile_pool(name="hp", bufs=2) as hp,
            tc.tile_pool(name="yp", bufs=3) as yp,
            tc.tile_pool(name="sp", bufs=2) as sp,
            tc.tile_pool(name="ps", bufs=2, space="PSUM") as ps,
            tc.tile_pool(name="psz", bufs=2, space="PSUM") as psz,
        ):
            w1_sb = const.tile([D, H], F32R, tag="w1")
            nc.sync.dma_start(out=w1_sb, in_=w1[:])
            w2_sb = const.tile([H, DT], F32R, tag="w2")
            nc.sync.dma_start(out=w2_sb, in_=w2[:])
            c1_sb = const.tile([D, H], F32R, tag="c1m")
            nc.sync.dma_start(out=c1_sb, in_=c1m[:])
            c2_sb = const.tile([H, GROUP + 3], F32R, tag="c2p")
            nc.sync.dma_start(out=c2_sb, in_=c2p[:])
            b1_sb = const.tile([H, 1], F32, tag="b1c")
            nc.sync.dma_start(out=b1_sb, in_=b1c[:])
            c1c_sb = const.tile([H, 1], F32, tag="c1c")
            nc.sync.dma_start(out=c1c_sb, in_=c1c[:])
            b2_sb = const.tile([DT, 1], F32, tag="b2c")
            nc.sync.dma_start(out=b2_sb, in_=b2c[:])
            c2c_sb = const.tile([GROUP, 1], F32, tag="c2c")
            nc.sync.dma_start(out=c2c_sb, in_=c2c[:])
            sc_all = const.tile([GROUP, nsc], F32, tag="sc")
            nc.sync.dma_start(out=sc_all, in_=scoresG[:])
            os_all = const.tile([GROUP, nsc], F32, tag="os")

            # item_prob values: sigmoid(max_t type_matching[s, t]) per type
            tm_sb = const.tile([S, T_TYPES], F32, tag="tm")
            nc.sync.dma_start(out=tm_sb, in_=tm[:])
            rmax = const.tile([S, 1], F32, tag="rmax")
            nc.vector.reduce_max(rmax[:], tm_sb[:], axis=mybir.AxisListType.X)
            prob_sb = const.tile([S, 1], F32, tag="prob")
            nc.scalar.activation(out=prob_sb, in_=rmax, func=SIGMOID)
            nc.sync.dma_start(out=probs[:], in_=prob_sb)

            for _rep in range(repeat):
              for g in range(ng):
                k = min(GROUP, n_tiles - g * GROUP)
                x_sb = xp.tile([P, GROUP * TILE], F32R, tag="x")
                nc.sync.dma_start(
                    out=x_sb[:, : k * TILE],
                    in_=statesT[:, g * GROUP * TILE : (g * GROUP + k) * TILE],
                )
                y_sb = yp.tile([DT, GROUP * TILE], F32, tag="y")
                z_ps = psz.tile([GROUP, TILE], F32, tag="z")
                for j in range(k):
                    xt = x_sb[:, j * TILE : (j + 1) * TILE]
                    h_ps = ps.tile([H, TILE], F32, tag="h")
                    nc.tensor.matmul(h_ps, w1_sb, xt, start=True, stop=True)
                    h_sb = hp.tile([H, TILE], F32R, tag="h")
                    nc.scalar.activation(out=h_sb, in_=h_ps, func=RELU, bias=b1_sb)
                    y_ps = ps.tile([DT, TILE], F32, tag="yps")
                    nc.tensor.matmul(y_ps, w2_sb, h_sb, start=True, stop=True)
                    nc.vector.tensor_scalar_add(
                        y_sb[:, j * TILE : (j + 1) * TILE], y_ps, b2_sb
                    )
                    hc_ps = ps.tile([H, TILE], F32, tag="hc")
                    nc.tensor.matmul(hc_ps, c1_sb, xt, start=True, stop=True)
                    hc_sb = hp.tile([H, TILE], F32R, tag="hc")
                    nc.scalar.activation(out=hc_sb, in_=hc_ps, func=RELU, bias=c1c_sb)
                    nc.tensor.matmul(
                        z_ps,
                        c2_sb[:, GROUP - 1 - j : 2 * GROUP - 1 - j],
                        hc_sb,
                        start=(j == 0),
                        stop=(j == k - 1),
                    )
                nc.sync.dma_start(
                    out=outT[:, g * GROUP * TILE : (g * GROUP + k) * TILE],
                    in_=y_sb[:, : k * TILE],
                )
                zs_sb = sp.tile([GROUP, TILE], F32, tag="zs")
                nc.scalar.activation(out=zs_sb, in_=z_ps, func=SIGMOID, bias=c2c_sb)
                nc.vector.tensor_tensor(
                    os_all[:, g * TILE : (g + 1) * TILE],
                    zs_sb,
                    sc_all[:, g * TILE : (g + 1) * TILE],
                    mybir.AluOpType.min,
                )
            nc.sync.dma_start(out=oscore[:], in_=os_all)

    nc.finalize()
    return nc


def _shard(states, scores, type_ids, type_matching, W1, b1, W2, b2, C1, c1, C2, c2):
    """Host-side sharding: route items by type to cores, build per-core inputs."""
    tm = np.ascontiguousarray(np.asarray(type_matching, dtype=np.float32))
    best_t = np.argmax(tm, axis=1)
    tids = np.asarray(type_ids)
    states = np.asarray(states, dtype=np.float32)
    scores_flat = np.asarray(scores, dtype=np.float32).reshape(-1)

    core_idx = []
    for s in range(S):
        idx = np.flatnonzero(tids == s)
        h = (len(idx) + 1) // 2
        core_idx.append(idx[:h])
        core_idx.append(idx[h:])

    n_tiles = max(1, max((len(ci) + TILE - 1) // TILE for ci in core_idx))
    ng = (n_tiles + GROUP - 1) // GROUP
    npad = n_tiles * TILE
    nsc = ng * TILE

    in_maps = []
    for c in range(N_CORES):
        s = c // 2
        t = int(best_t[s])
        ci = core_idx[c]
        cip = np.zeros(npad, dtype=np.int64)
        cip[: len(ci)] = ci
        stT = np.ascontiguousarray(states[cip].T)
        scp = np.zeros(GROUP * nsc, dtype=np.float32)
        scp[:npad] = scores_flat[cip]
        scG = np.ascontiguousarray(
            scp.reshape(ng, GROUP, TILE).transpose(1, 0, 2).reshape(GROUP, nsc)
        )
        c2pad = np.zeros((H, GROUP + 3), dtype=np.float32)
        c2pad[:, GROUP - 1] = np.asarray(C2[s, t], dtype=np.float32).reshape(-1)
        in_maps.append(
            {
                "statesT": stT,
                "scoresG": scG,
                "w1": np.ascontiguousarray(np.asarray(W1[s, t], dtype=np.float32)),
                "w2": np.ascontiguousarray(np.asarray(W2[s, t], dtype=np.float32)),
                "c1m": np.ascontiguousarray(np.asarray(C1[s, t], dtype=np.float32)),
                "c2p": c2pad,
                "b1c": np.asarray(b1[s, t], dtype=np.float32).reshape(H, 1),
                "c1c": np.asarray(c1[s, t], dtype=np.float32).reshape(H, 1),
                "b2c": np.asarray(b2[s, t], dtype=np.float32).reshape(DT, 1),
                "c2c": np.full(
                    (GROUP, 1), np.float32(np.asarray(c2[s, t]).reshape(())),
                    dtype=np.float32,
                ),
                "tm": tm,
            }
        )
    return in_maps, core_idx, n_tiles


def _unshard(results, core_idx, n_tiles):
    ng = (n_tiles + GROUP - 1) // GROUP
    nsc = ng * TILE
    out_state = np.zeros((N, DT), dtype=np.float32)
    out_score = np.zeros((N, 1), dtype=np.float32)
    item_prob = np.zeros((N,), dtype=np.float32)
    for c in range(N_CORES):
        ci = core_idx[c]
        n = len(ci)
        if n == 0:
            continue
        r = results[c]
        out_state[ci] = r["outT"][:, :n].T
        osg = (
            r["oscore"].reshape(GROUP, ng, TILE).transpose(1, 0, 2).reshape(-1)[:n]
        )
        out_score[ci, 0] = osg
        item_prob[ci] = r["probs"][c // 2, 0]
    return out_state, out_score, item_prob


_NC_CACHE: dict[int, object] = {}


def kernel(states, scores, type_ids, type_matching, W1, b1, W2, b2, C1, c1, C2, c2):
    in_maps, core_idx, n_tiles = _shard(
        states, scores, type_ids, type_matching, W1, b1, W2, b2, C1, c1, C2, c2
    )
    nc = _NC_CACHE.get((n_tiles, 1))
    if nc is None:
        nc = build_bass(n_tiles)
        _NC_CACHE[(n_tiles, 1)] = nc
    res = run_bass_kernel_spmd(nc, in_maps, core_ids=list(range(N_CORES)))
    return _unshard(res.results, core_idx, n_tiles)
